# revision 13
# baseline (speedup 1.0000x reference)
"""Trainium2 Bass kernel for nn_MixQuantize (vq_codebook).

Math (per position (b, l), over NE=8192 codes):
  logits = proj_w @ z + proj_b                       (1x1 conv)
  mask   = per-batch random half split from mask_noise ranks
  ind    = argmax_n(logits + gumbel)  if masked else argmax_n(logits)
  z_q    = embed_w[ind]
  prior  = KLW * mean( sum_n softmax(logits) * log(softmax(logits)*NE) )

Device strategy (8 NeuronCores, data-parallel over batch, 2 batches/core):
  Layout: positions on partitions, codes on the free axis.  The host
  permutes each batch's 256 positions so the 128 masked ("stoch")
  positions form one 128-row block and the 128 unmasked ("det")
  positions another — gumbel is only fetched for masked rows (halves
  its traffic) and each row-block needs exactly one argmax flavor.
  logits are computed as 3 bf16-split matmuls (z0w0+z0w1+z1w0) which
  reproduces the fp32 reference argmax with ~40x margin on top-2 gaps.
  ScalarE does exp(a-mhat) with a running per-partition accumulator
  (S0); VectorE scalar_tensor_tensor computes a*exp with accumulator
  (S1); gumbel is added into the logits copy by the DMA engine
  (software-DGE accumulate).  argmax = vector.max + vector.max_index.
  The codebook gather, KL reduction and un-permutation run on host.
"""

import numpy as np
import ml_dtypes

import concourse.bass as bass
import concourse.bacc as bacc
import concourse.tile as tile
from concourse import mybir
from concourse.bass_utils import run_bass_kernel_spmd

B, NH, NE, ED, HWD = 16, 256, 8192, 256, 16
L = HWD * HWD
LEN_KEEP = L // 2
KLW = 0.0005
N_CORES = 8
BPC = B // N_CORES          # batches per core
N_RB = 2 * BPC              # row blocks per core (stoch+det per batch)
CHUNK = 1024                # free-dim chunk (2 PSUM banks)
N_CHUNK = NE // CHUNK

_bf16 = ml_dtypes.bfloat16


def _build(exp_bias: float, with_bias: bool):
    nc = bacc.Bacc(trn_type="TRN2", debug=False, target_bir_lowering=False,
                   num_devices=N_CORES)
    f32, bf16, u16, u32 = (mybir.dt.float32, mybir.dt.bfloat16,
                           mybir.dt.uint16, mybir.dt.uint32)

    z0 = nc.dram_tensor("z0", [128, 2, BPC, L], bf16, kind="ExternalInput").ap()
    w0t = nc.dram_tensor("w0t", [128, 2, NE], bf16, kind="ExternalInput").ap()
    gp = nc.dram_tensor("gp", [BPC, 128, NE], f32, kind="ExternalInput").ap()
    if with_bias:
        pb0 = nc.dram_tensor("pb0", [1, NE], bf16, kind="ExternalInput").ap()
        pb1 = nc.dram_tensor("pb1", [1, NE], bf16, kind="ExternalInput").ap()

    idx_out = nc.dram_tensor("idx_out", [N_RB, 128, N_CHUNK, 8], u32,
                             kind="ExternalOutput").ap()
    st_out = nc.dram_tensor("st_out", [N_RB, 2, 128], f32,
                            kind="ExternalOutput").ap()

    with tile.TileContext(nc) as tc:
        with (
            tc.tile_pool(name="big", bufs=1) as big,
            tc.tile_pool(name="ps", bufs=4, space="PSUM") as ps,
            tc.tile_pool(name="ck", bufs=8) as ck,
            tc.tile_pool(name="sm", bufs=2) as sm,
            tc.tile_pool(name="st", bufs=4) as st,
        ):
            # z first (small, needed by the first matmul), then weight
            # columns chunked over both HWDGE queues so compute starts
            # as soon as the first chunk lands
            z0_sb = big.tile([128, 2, BPC, L], bf16)
            nc.sync.dma_start(out=z0_sb, in_=z0)
            w0_sb = big.tile([128, 2, NE], bf16)
            _qs = [nc.sync, nc.scalar]
            for q in range(CHUNK // 512):
                qsl = slice(q * 512, (q + 1) * 512)
                _qs[q % 2].dma_start(out=w0_sb[:, :, qsl], in_=w0t[:, :, qsl])
            for ch in range(1, N_CHUNK):
                csl = slice(ch * CHUNK, (ch + 1) * CHUNK)
                _qs[ch % 2].dma_start(out=w0_sb[:, :, csl], in_=w0t[:, :, csl])
            if with_bias:
                pb0_sb = big.tile([1, NE], bf16)
                nc.sync.dma_start(out=pb0_sb, in_=pb0)
                pb1_sb = big.tile([1, NE], bf16)
                nc.sync.dma_start(out=pb1_sb, in_=pb1)
                ones_sb = big.tile([1, 128], bf16)
                nc.vector.memset(ones_sb, 1.0)
            ebias = big.tile([128, 1], f32)
            nc.vector.memset(ebias, -float(exp_bias))

            for rb in range(N_RB):
                k, half = rb // 2, rb % 2      # batch-local, 0=stoch 1=det
                stoch = (half == 0)
                lsl = slice(half * 128, (half + 1) * 128)

                s0c = st.tile([128, N_CHUNK], f32, tag="s0c")
                s1c = st.tile([128, N_CHUNK], f32, tag="s1c")
                m8c = st.tile([128, N_CHUNK, 8], f32, tag="m8c")
                i8c = st.tile([128, N_CHUNK, 8], u32, tag="i8c")

                r_cks = []
                for ch in range(N_CHUNK):
                    csl = slice(ch * CHUNK, (ch + 1) * CHUNK)
                    a_ps = ps.tile([128, CHUNK], f32)
                    # lhsT-stationary ordering: LDWEIGHTS shared across wj/j4
                    groups = [(0, z0_sb, (w0_sb,)),
                              (1, z0_sb, (w0_sb,))]
                    n_sub = CHUNK // 512
                    nmm_slice = 2 + (2 if with_bias else 0)
                    cnt = [0] * n_sub
                    for (kk, zz, wws) in groups:
                        for ww in wws:
                            for j4 in range(n_sub):
                                nsl = slice(ch * CHUNK + j4 * 512,
                                            ch * CHUNK + (j4 + 1) * 512)
                                psl = slice(j4 * 512, (j4 + 1) * 512)
                                nc.tensor.matmul(
                                    a_ps[:, psl],
                                    zz[:, kk, k, lsl],
                                    ww[:, kk, nsl],
                                    start=(cnt[j4] == 0),
                                    stop=(cnt[j4] == nmm_slice - 1))
                                cnt[j4] += 1
                    if with_bias:
                        for pbs in (pb0_sb, pb1_sb):
                            for j4 in range(n_sub):
                                nsl = slice(ch * CHUNK + j4 * 512,
                                            ch * CHUNK + (j4 + 1) * 512)
                                psl = slice(j4 * 512, (j4 + 1) * 512)
                                nc.tensor.matmul(
                                    a_ps[:, psl], ones_sb, pbs[:, nsl],
                                    start=False,
                                    stop=(cnt[j4] == nmm_slice - 1))
                                cnt[j4] += 1
                    # exp(a - mhat) with running S0 accumulator
                    e_ck = ck.tile([128, CHUNK], bf16, tag="e_ck")
                    nc.scalar.activation(e_ck, a_ps,
                                         mybir.ActivationFunctionType.Exp,
                                         bias=ebias[:, 0:1], scale=1.0,
                                         accum_out=s0c[:, ch:ch + 1])
                    # S1 accumulator: sum a * exp(a - mhat)
                    scr = ck.tile([128, CHUNK], bf16, tag="scr")
                    nc.vector.scalar_tensor_tensor(
                        out=scr, in0=a_ps, scalar=0.0, in1=e_ck,
                        op0=mybir.AluOpType.add, op1=mybir.AluOpType.mult,
                        accum_out=s1c[:, ch:ch + 1])
                    # value row chunk: a (det rows scan PSUM directly) or
                    # a+gumbel (stoch; the accumulate-DMA gets two chunks of
                    # slack before the deferred scans read it)
                    if stoch:
                        r_ck = ck.tile([128, CHUNK], f32, tag="r_ck")
                        nc.scalar.activation(r_ck, a_ps,
                                             mybir.ActivationFunctionType.Copy)
                        nc.gpsimd.dma_start(out=r_ck, in_=gp[k, :, csl],
                                            accum_op=mybir.AluOpType.add)
                    else:
                        r_ck = a_ps
                    r_cks.append(r_ck)
                    # deferred by two chunks: top-8 + indices of chunk ch-2
                    if ch > 1:
                        pc = ch - 2
                        nc.vector.max(m8c[:, pc], r_cks[pc])
                        nc.vector.max_index(i8c[:, pc], m8c[:, pc], r_cks[pc])

                for pc in (N_CHUNK - 2, N_CHUNK - 1):
                    nc.vector.max(m8c[:, pc], r_cks[pc])
                    nc.vector.max_index(i8c[:, pc], m8c[:, pc], r_cks[pc])

                nc.sync.dma_start(out=idx_out[rb], in_=i8c)
                s0r = sm.tile([128, 1], f32, tag="s0r")
                nc.vector.reduce_sum(s0r, s0c, axis=mybir.AxisListType.X)
                nc.sync.dma_start(out=st_out[rb, 0], in_=s0r)
                s1r = sm.tile([128, 1], f32, tag="s1r")
                nc.vector.reduce_sum(s1r, s1c, axis=mybir.AxisListType.X)
                nc.sync.dma_start(out=st_out[rb, 1], in_=s1r)
    nc.compile()
    return nc


_NC_CACHE = {}


def kernel(z, proj_w, proj_b, embed_w, gumbel, mask_noise, _trace=False):
    z = np.asarray(z, dtype=np.float32)
    proj_w = np.asarray(proj_w, dtype=np.float32)
    proj_b = np.asarray(proj_b, dtype=np.float32)
    embed_w = np.asarray(embed_w, dtype=np.float32)
    gumbel = np.asarray(gumbel, dtype=np.float32)
    mask_noise = np.asarray(mask_noise, dtype=np.float32)

    # --- host prep -------------------------------------------------------
    # mask, replicating jnp.argsort (stable) semantics exactly
    ids_shuffle = np.argsort(mask_noise, axis=1, kind="stable")
    ids_restore = np.argsort(ids_shuffle, axis=1, kind="stable")
    base = (np.arange(L) >= LEN_KEEP)
    mask = np.take_along_axis(np.broadcast_to(base, (B, L)), ids_restore,
                              axis=1)                      # (B, L) bool
    # per-batch position permutation: masked first, unmasked second
    perm = np.empty((B, L), dtype=np.int64)
    for b in range(B):
        m = mask[b]
        perm[b] = np.concatenate([np.where(m)[0], np.where(~m)[0]])

    zf = z.reshape(B, NH, L)
    zp = np.take_along_axis(zf, perm[:, None, :], axis=2)  # (B, NH, L)
    z0 = zp.astype(_bf16)
    def _zlay(a):  # (BPC', NH, L) -> (128, 2, BPC', L)
        return np.ascontiguousarray(
            a.reshape(a.shape[0], 2, 128, L).transpose(2, 1, 0, 3))

    w0 = proj_w.astype(_bf16)
    def _wlay(w):  # (NE, NH) -> (128, 2, NE)
        return np.ascontiguousarray(w.T.reshape(2, 128, NE).transpose(1, 0, 2))
    w0t = _wlay(w0)

    gf = gumbel.reshape(B, NE, L)
    gperm = np.empty((B, 128, NE), dtype=np.float32)
    for b in range(B):
        gperm[b] = gf[b][:, perm[b][:LEN_KEEP]].T          # (128, NE)

    with_bias = bool(np.any(proj_b != 0.0))
    pb0 = proj_b.astype(_bf16)[None, :]
    pb1 = (proj_b - pb0[0].astype(np.float32)).astype(_bf16)[None, :]

    # exp bias: safe upper bound on |logits| (Cauchy-Schwarz), rounded for
    # program-cache stability
    zn = np.sqrt((zf.astype(np.float64) ** 2).sum(axis=1)).max()
    wn = np.sqrt((proj_w.astype(np.float64) ** 2).sum(axis=1)).max()
    mhat = float(np.ceil(zn * wn + np.abs(proj_b).max() + 1.0))

    key = (mhat, with_bias)
    if key not in _NC_CACHE:
        _NC_CACHE[key] = _build(mhat, with_bias)
    nc = _NC_CACHE[key]

    in_maps = []
    for c in range(N_CORES):
        bs = slice(c * BPC, (c + 1) * BPC)
        m = dict(z0=_zlay(z0[bs]), w0t=w0t, gp=gperm[bs])
        if with_bias:
            m["pb0"] = pb0
            m["pb1"] = pb1
        in_maps.append(m)

    res = run_bass_kernel_spmd(nc, in_maps, list(range(N_CORES)),
                               trace=_trace)

    # --- host assembly ---------------------------------------------------
    ind = np.empty((B, L), dtype=np.int32)
    s0_all = np.empty((B, L), dtype=np.float64)
    s1_all = np.empty((B, L), dtype=np.float64)
    for c in range(N_CORES):
        r = res.results[c]
        for k in range(BPC):
            b = c * BPC + k
            # exact rescore of the stage-1 candidates (<=64 per position)
            chunk_base = (np.arange(N_CHUNK, dtype=np.int64) * CHUNK)[None, :, None]
            parts = []
            for li, rbl in ((0, 2 * k), (1, 2 * k + 1)):
                cand = (r["idx_out"][rbl].astype(np.int64)
                        + chunk_base).reshape(128, N_CHUNK * 8)
                wc = proj_w[cand]                    # (128, 64, NH) f32
                zc = zp[b, :, li * 128:(li + 1) * 128]       # (NH, 128)
                vals = np.einsum('pkc,cp->pk', wc.astype(np.float64),
                                 zc.astype(np.float64))
                vals += proj_b.astype(np.float64)[cand]
                if li == 0:                          # stoch: add gumbel
                    lpos = perm[b][:LEN_KEEP]
                    vals += gf[b][cand, lpos[:, None]].astype(np.float64)
                best = vals.argmax(axis=1)
                parts.append(cand[np.arange(128), best])
            ind[b, perm[b]] = np.concatenate(parts).astype(np.int32)
            s0p = np.concatenate([r["st_out"][2 * k, 0],
                                  r["st_out"][2 * k + 1, 0]])
            s1p = np.concatenate([r["st_out"][2 * k, 1],
                                  r["st_out"][2 * k + 1, 1]])
            s0_all[b, perm[b]] = s0p
            s1_all[b, perm[b]] = s1p

    kl = s1_all / s0_all - (mhat + np.log(s0_all)) + np.log(NE)
    prior_loss = np.float32(KLW * kl.mean())

    z_q = embed_w[ind]                                     # (B, L, ED)
    z_q = np.ascontiguousarray(z_q.transpose(0, 2, 1)).reshape(B, ED, HWD, HWD)

    ind = ind.reshape(B, HWD, HWD)
    if _trace:
        return (z_q, prior_loss, ind), res
    return z_q, prior_loss, ind


# revision 14
# speedup vs baseline: 1.1018x; 1.1018x over previous
"""Trainium2 Bass kernel for nn_MixQuantize (vq_codebook).

Math (per position (b, l), over NE=8192 codes):
  logits = proj_w @ z + proj_b                       (1x1 conv)
  mask   = per-batch random half split from mask_noise ranks
  ind    = argmax_n(logits + gumbel)  if masked else argmax_n(logits)
  z_q    = embed_w[ind]
  prior  = KLW * mean( sum_n softmax(logits) * log(softmax(logits)*NE) )

Device strategy (8 NeuronCores, data-parallel over batch, 2 batches/core):
  Layout: positions on partitions, codes on the free axis.  The host
  permutes each batch's 256 positions so the 128 masked ("stoch")
  positions form one 128-row block and the 128 unmasked ("det")
  positions another — gumbel is only fetched for masked rows (halves
  its traffic) and each row-block needs exactly one argmax flavor.
  logits are computed as 3 bf16-split matmuls (z0w0+z0w1+z1w0) which
  reproduces the fp32 reference argmax with ~40x margin on top-2 gaps.
  ScalarE does exp(a-mhat) with a running per-partition accumulator
  (S0); VectorE scalar_tensor_tensor computes a*exp with accumulator
  (S1); gumbel is added into the logits copy by the DMA engine
  (software-DGE accumulate).  argmax = vector.max + vector.max_index.
  The codebook gather, KL reduction and un-permutation run on host.
"""

import numpy as np
import ml_dtypes

import concourse.bass as bass
import concourse.bacc as bacc
import concourse.tile as tile
from concourse import mybir
from concourse.bass_utils import run_bass_kernel_spmd

B, NH, NE, ED, HWD = 16, 256, 8192, 256, 16
L = HWD * HWD
LEN_KEEP = L // 2
KLW = 0.0005
N_CORES = 8
BPC = B // N_CORES          # batches per core
N_RB = 2 * BPC              # row blocks per core (stoch+det per batch)
CHUNK = 1024                # free-dim chunk (2 PSUM banks)
N_CHUNK = NE // CHUNK

_bf16 = ml_dtypes.bfloat16


def _build(exp_bias: float, with_bias: bool):
    nc = bacc.Bacc(trn_type="TRN2", debug=False, target_bir_lowering=False,
                   num_devices=N_CORES)
    f32, bf16, u16, u32 = (mybir.dt.float32, mybir.dt.bfloat16,
                           mybir.dt.uint16, mybir.dt.uint32)

    z0 = nc.dram_tensor("z0", [128, 2, BPC, L], bf16, kind="ExternalInput").ap()
    w0t = nc.dram_tensor("w0t", [128, 2, NE], bf16, kind="ExternalInput").ap()
    gp = nc.dram_tensor("gp", [BPC, 128, NE], f32, kind="ExternalInput").ap()
    if with_bias:
        pb0 = nc.dram_tensor("pb0", [1, NE], bf16, kind="ExternalInput").ap()
        pb1 = nc.dram_tensor("pb1", [1, NE], bf16, kind="ExternalInput").ap()

    idx_out = nc.dram_tensor("idx_out", [N_RB, 128, N_CHUNK, 8], u32,
                             kind="ExternalOutput").ap()
    st_out = nc.dram_tensor("st_out", [N_RB, 2, 128], f32,
                            kind="ExternalOutput").ap()

    with tile.TileContext(nc) as tc:
        with (
            tc.tile_pool(name="big", bufs=1) as big,
            tc.tile_pool(name="ps", bufs=4, space="PSUM") as ps,
            tc.tile_pool(name="ck", bufs=8) as ck,
            tc.tile_pool(name="sm", bufs=2) as sm,
            tc.tile_pool(name="st", bufs=4) as st,
        ):
            # z first (small, needed by the first matmul), then weight
            # columns chunked over both HWDGE queues so compute starts
            # as soon as the first chunk lands
            z0_sb = big.tile([128, 2, BPC, L], bf16)
            nc.sync.dma_start(out=z0_sb, in_=z0)
            w0_sb = big.tile([128, 2, NE], bf16)
            _qs = [nc.sync, nc.scalar]
            for q in range(CHUNK // 512):
                qsl = slice(q * 512, (q + 1) * 512)
                _qs[q % 2].dma_start(out=w0_sb[:, :, qsl], in_=w0t[:, :, qsl])
            for ch in range(1, N_CHUNK):
                csl = slice(ch * CHUNK, (ch + 1) * CHUNK)
                _qs[ch % 2].dma_start(out=w0_sb[:, :, csl], in_=w0t[:, :, csl])
            if with_bias:
                pb0_sb = big.tile([1, NE], bf16)
                nc.sync.dma_start(out=pb0_sb, in_=pb0)
                pb1_sb = big.tile([1, NE], bf16)
                nc.sync.dma_start(out=pb1_sb, in_=pb1)
                ones_sb = big.tile([1, 128], bf16)
                nc.vector.memset(ones_sb, 1.0)
            ebias = big.tile([128, 1], f32)
            nc.vector.memset(ebias, -float(exp_bias))

            for rb in range(N_RB):
                k, half = rb // 2, rb % 2      # batch-local, 0=stoch 1=det
                stoch = (half == 0)
                lsl = slice(half * 128, (half + 1) * 128)

                s0c = st.tile([128, N_CHUNK], f32, tag="s0c")
                s1c = st.tile([128, N_CHUNK], f32, tag="s1c")
                m8c = st.tile([128, N_CHUNK, 8], f32, tag="m8c")
                i8c = st.tile([128, N_CHUNK, 8], u32, tag="i8c")

                r_cks = []
                for ch in range(N_CHUNK):
                    csl = slice(ch * CHUNK, (ch + 1) * CHUNK)
                    a_ps = ps.tile([128, CHUNK], f32)
                    # lhsT-stationary ordering: LDWEIGHTS shared across wj/j4
                    groups = [(0, z0_sb, (w0_sb,)),
                              (1, z0_sb, (w0_sb,))]
                    n_sub = CHUNK // 512
                    nmm_slice = 2 + (2 if with_bias else 0)
                    cnt = [0] * n_sub
                    for (kk, zz, wws) in groups:
                        for ww in wws:
                            for j4 in range(n_sub):
                                nsl = slice(ch * CHUNK + j4 * 512,
                                            ch * CHUNK + (j4 + 1) * 512)
                                psl = slice(j4 * 512, (j4 + 1) * 512)
                                nc.tensor.matmul(
                                    a_ps[:, psl],
                                    zz[:, kk, k, lsl],
                                    ww[:, kk, nsl],
                                    start=(cnt[j4] == 0),
                                    stop=(cnt[j4] == nmm_slice - 1))
                                cnt[j4] += 1
                    if with_bias:
                        for pbs in (pb0_sb, pb1_sb):
                            for j4 in range(n_sub):
                                nsl = slice(ch * CHUNK + j4 * 512,
                                            ch * CHUNK + (j4 + 1) * 512)
                                psl = slice(j4 * 512, (j4 + 1) * 512)
                                nc.tensor.matmul(
                                    a_ps[:, psl], ones_sb, pbs[:, nsl],
                                    start=False,
                                    stop=(cnt[j4] == nmm_slice - 1))
                                cnt[j4] += 1
                    # value row chunk first: a (det rows scan PSUM
                    # directly) or a+gumbel (stoch; accumulate-DMA gets a
                    # full chunk of slack before the deferred scans read it)
                    if stoch:
                        r_ck = ck.tile([128, CHUNK], f32, tag="r_ck")
                        nc.scalar.activation(r_ck, a_ps,
                                             mybir.ActivationFunctionType.Copy)
                        nc.gpsimd.dma_start(out=r_ck, in_=gp[k, :, csl],
                                            accum_op=mybir.AluOpType.add)
                    else:
                        r_ck = a_ps
                    # exp(a - mhat) with running S0 accumulator
                    e_ck = ck.tile([128, CHUNK], bf16, tag="e_ck")
                    nc.scalar.activation(e_ck, a_ps,
                                         mybir.ActivationFunctionType.Exp,
                                         bias=ebias[:, 0:1], scale=1.0,
                                         accum_out=s0c[:, ch:ch + 1])
                    # S1 accumulator: sum a * exp(a - mhat)
                    scr = ck.tile([128, CHUNK], bf16, tag="scr")
                    nc.vector.scalar_tensor_tensor(
                        out=scr, in0=a_ps, scalar=0.0, in1=e_ck,
                        op0=mybir.AluOpType.add, op1=mybir.AluOpType.mult,
                        accum_out=s1c[:, ch:ch + 1])
                    r_cks.append(r_ck)
                    # deferred by one chunk: top-8 + indices of chunk ch-1
                    if ch > 0:
                        pc = ch - 1
                        nc.vector.max(m8c[:, pc], r_cks[pc])
                        nc.vector.max_index(i8c[:, pc], m8c[:, pc], r_cks[pc])

                pc = N_CHUNK - 1
                nc.vector.max(m8c[:, pc], r_cks[pc])
                nc.vector.max_index(i8c[:, pc], m8c[:, pc], r_cks[pc])

                nc.sync.dma_start(out=idx_out[rb], in_=i8c)
                s0r = sm.tile([128, 1], f32, tag="s0r")
                nc.vector.reduce_sum(s0r, s0c, axis=mybir.AxisListType.X)
                nc.sync.dma_start(out=st_out[rb, 0], in_=s0r)
                s1r = sm.tile([128, 1], f32, tag="s1r")
                nc.vector.reduce_sum(s1r, s1c, axis=mybir.AxisListType.X)
                nc.sync.dma_start(out=st_out[rb, 1], in_=s1r)
    nc.compile()
    return nc


_NC_CACHE = {}


def kernel(z, proj_w, proj_b, embed_w, gumbel, mask_noise, _trace=False):
    z = np.asarray(z, dtype=np.float32)
    proj_w = np.asarray(proj_w, dtype=np.float32)
    proj_b = np.asarray(proj_b, dtype=np.float32)
    embed_w = np.asarray(embed_w, dtype=np.float32)
    gumbel = np.asarray(gumbel, dtype=np.float32)
    mask_noise = np.asarray(mask_noise, dtype=np.float32)

    # --- host prep -------------------------------------------------------
    # mask, replicating jnp.argsort (stable) semantics exactly
    ids_shuffle = np.argsort(mask_noise, axis=1, kind="stable")
    ids_restore = np.argsort(ids_shuffle, axis=1, kind="stable")
    base = (np.arange(L) >= LEN_KEEP)
    mask = np.take_along_axis(np.broadcast_to(base, (B, L)), ids_restore,
                              axis=1)                      # (B, L) bool
    # per-batch position permutation: masked first, unmasked second
    perm = np.empty((B, L), dtype=np.int64)
    for b in range(B):
        m = mask[b]
        perm[b] = np.concatenate([np.where(m)[0], np.where(~m)[0]])

    zf = z.reshape(B, NH, L)
    zp = np.take_along_axis(zf, perm[:, None, :], axis=2)  # (B, NH, L)
    z0 = zp.astype(_bf16)
    def _zlay(a):  # (BPC', NH, L) -> (128, 2, BPC', L)
        return np.ascontiguousarray(
            a.reshape(a.shape[0], 2, 128, L).transpose(2, 1, 0, 3))

    w0 = proj_w.astype(_bf16)
    def _wlay(w):  # (NE, NH) -> (128, 2, NE)
        return np.ascontiguousarray(w.T.reshape(2, 128, NE).transpose(1, 0, 2))
    w0t = _wlay(w0)

    gf = gumbel.reshape(B, NE, L)
    gperm = np.empty((B, 128, NE), dtype=np.float32)
    for b in range(B):
        gperm[b] = gf[b][:, perm[b][:LEN_KEEP]].T          # (128, NE)

    with_bias = bool(np.any(proj_b != 0.0))
    pb0 = proj_b.astype(_bf16)[None, :]
    pb1 = (proj_b - pb0[0].astype(np.float32)).astype(_bf16)[None, :]

    # exp bias: safe upper bound on |logits| (Cauchy-Schwarz), rounded for
    # program-cache stability
    zn = np.sqrt((zf.astype(np.float64) ** 2).sum(axis=1)).max()
    wn = np.sqrt((proj_w.astype(np.float64) ** 2).sum(axis=1)).max()
    mhat = float(np.ceil(zn * wn + np.abs(proj_b).max() + 1.0))

    key = (mhat, with_bias)
    if key not in _NC_CACHE:
        _NC_CACHE[key] = _build(mhat, with_bias)
    nc = _NC_CACHE[key]

    in_maps = []
    for c in range(N_CORES):
        bs = slice(c * BPC, (c + 1) * BPC)
        m = dict(z0=_zlay(z0[bs]), w0t=w0t, gp=gperm[bs])
        if with_bias:
            m["pb0"] = pb0
            m["pb1"] = pb1
        in_maps.append(m)

    res = run_bass_kernel_spmd(nc, in_maps, list(range(N_CORES)),
                               trace=_trace)

    # --- host assembly ---------------------------------------------------
    ind = np.empty((B, L), dtype=np.int32)
    s0_all = np.empty((B, L), dtype=np.float64)
    s1_all = np.empty((B, L), dtype=np.float64)
    for c in range(N_CORES):
        r = res.results[c]
        for k in range(BPC):
            b = c * BPC + k
            # exact rescore of the stage-1 candidates (<=64 per position)
            chunk_base = (np.arange(N_CHUNK, dtype=np.int64) * CHUNK)[None, :, None]
            parts = []
            for li, rbl in ((0, 2 * k), (1, 2 * k + 1)):
                cand = (r["idx_out"][rbl].astype(np.int64)
                        + chunk_base).reshape(128, N_CHUNK * 8)
                wc = proj_w[cand]                    # (128, 64, NH) f32
                zc = zp[b, :, li * 128:(li + 1) * 128]       # (NH, 128)
                vals = np.einsum('pkc,cp->pk', wc.astype(np.float64),
                                 zc.astype(np.float64))
                vals += proj_b.astype(np.float64)[cand]
                if li == 0:                          # stoch: add gumbel
                    lpos = perm[b][:LEN_KEEP]
                    vals += gf[b][cand, lpos[:, None]].astype(np.float64)
                best = vals.argmax(axis=1)
                parts.append(cand[np.arange(128), best])
            ind[b, perm[b]] = np.concatenate(parts).astype(np.int32)
            s0p = np.concatenate([r["st_out"][2 * k, 0],
                                  r["st_out"][2 * k + 1, 0]])
            s1p = np.concatenate([r["st_out"][2 * k, 1],
                                  r["st_out"][2 * k + 1, 1]])
            s0_all[b, perm[b]] = s0p
            s1_all[b, perm[b]] = s1p

    kl = s1_all / s0_all - (mhat + np.log(s0_all)) + np.log(NE)
    prior_loss = np.float32(KLW * kl.mean())

    z_q = embed_w[ind]                                     # (B, L, ED)
    z_q = np.ascontiguousarray(z_q.transpose(0, 2, 1)).reshape(B, ED, HWD, HWD)

    ind = ind.reshape(B, HWD, HWD)
    if _trace:
        return (z_q, prior_loss, ind), res
    return z_q, prior_loss, ind


# revision 15
# speedup vs baseline: 1.1362x; 1.0313x over previous
"""Trainium2 Bass kernel for nn_MixQuantize (vq_codebook).

Math (per position (b, l), over NE=8192 codes):
  logits = proj_w @ z + proj_b                       (1x1 conv)
  mask   = per-batch random half split from mask_noise ranks
  ind    = argmax_n(logits + gumbel)  if masked else argmax_n(logits)
  z_q    = embed_w[ind]
  prior  = KLW * mean( sum_n softmax(logits) * log(softmax(logits)*NE) )

Device strategy (8 NeuronCores, data-parallel over batch, 2 batches/core):
  Layout: positions on partitions, codes on the free axis.  The host
  permutes each batch's 256 positions so the 128 masked ("stoch")
  positions form one 128-row block and the 128 unmasked ("det")
  positions another — gumbel is only fetched for masked rows (halves
  its traffic) and each row-block needs exactly one argmax flavor.
  logits are computed as 3 bf16-split matmuls (z0w0+z0w1+z1w0) which
  reproduces the fp32 reference argmax with ~40x margin on top-2 gaps.
  ScalarE does exp(a-mhat) with a running per-partition accumulator
  (S0); VectorE scalar_tensor_tensor computes a*exp with accumulator
  (S1); gumbel is added into the logits copy by the DMA engine
  (software-DGE accumulate).  argmax = vector.max + vector.max_index.
  The codebook gather, KL reduction and un-permutation run on host.
"""

import numpy as np
import ml_dtypes

import concourse.bass as bass
import concourse.bacc as bacc
import concourse.tile as tile
from concourse import mybir
from concourse.bass_utils import run_bass_kernel_spmd

B, NH, NE, ED, HWD = 16, 256, 8192, 256, 16
L = HWD * HWD
LEN_KEEP = L // 2
KLW = 0.0005
N_CORES = 8
BPC = B // N_CORES          # batches per core
N_RB = 2 * BPC              # row blocks per core (stoch+det per batch)
CHUNK = 1024                # free-dim chunk (2 PSUM banks)
N_CHUNK = NE // CHUNK

_bf16 = ml_dtypes.bfloat16


def _build(exp_bias: float, with_bias: bool):
    nc = bacc.Bacc(trn_type="TRN2", debug=False, target_bir_lowering=False,
                   num_devices=N_CORES)
    f32, bf16, u16, u32 = (mybir.dt.float32, mybir.dt.bfloat16,
                           mybir.dt.uint16, mybir.dt.uint32)

    z0 = nc.dram_tensor("z0", [128, 2, BPC, L], bf16, kind="ExternalInput").ap()
    w0t = nc.dram_tensor("w0t", [128, 2, NE], bf16, kind="ExternalInput").ap()
    gp = nc.dram_tensor("gp", [BPC, 128, NE], f32, kind="ExternalInput").ap()
    if with_bias:
        pb0 = nc.dram_tensor("pb0", [1, NE], bf16, kind="ExternalInput").ap()
        pb1 = nc.dram_tensor("pb1", [1, NE], bf16, kind="ExternalInput").ap()

    idx_out = nc.dram_tensor("idx_out", [N_RB, 128, N_CHUNK, 8], u32,
                             kind="ExternalOutput").ap()
    st_out = nc.dram_tensor("st_out", [N_RB, 2, 128], f32,
                            kind="ExternalOutput").ap()

    with tile.TileContext(nc) as tc:
        with (
            tc.tile_pool(name="big", bufs=1) as big,
            tc.tile_pool(name="ps", bufs=4, space="PSUM") as ps,
            tc.tile_pool(name="ck", bufs=8) as ck,
            tc.tile_pool(name="sm", bufs=2) as sm,
            tc.tile_pool(name="st", bufs=4) as st,
        ):
            # z first (small, needed by the first matmul), then weight
            # columns chunked over both HWDGE queues so compute starts
            # as soon as the first chunk lands
            z0_sb = big.tile([128, 2, BPC, L], bf16)
            nc.sync.dma_start(out=z0_sb, in_=z0)
            w0_sb = big.tile([128, 2, NE], bf16)
            _qs = [nc.sync, nc.scalar]
            for q in range(CHUNK // 512):
                qsl = slice(q * 512, (q + 1) * 512)
                _qs[q % 2].dma_start(out=w0_sb[:, :, qsl], in_=w0t[:, :, qsl])
            for ch in range(1, N_CHUNK):
                csl = slice(ch * CHUNK, (ch + 1) * CHUNK)
                _qs[ch % 2].dma_start(out=w0_sb[:, :, csl], in_=w0t[:, :, csl])
            if with_bias:
                pb0_sb = big.tile([1, NE], bf16)
                nc.sync.dma_start(out=pb0_sb, in_=pb0)
                pb1_sb = big.tile([1, NE], bf16)
                nc.sync.dma_start(out=pb1_sb, in_=pb1)
                ones_sb = big.tile([1, 128], bf16)
                nc.vector.memset(ones_sb, 1.0)
            ebias = big.tile([128, 1], f32)
            nc.vector.memset(ebias, -float(exp_bias))
            # warm the ACT exp table before the first real chunk
            warm = big.tile([128, 1], f32)
            nc.scalar.activation(warm, ebias,
                                 mybir.ActivationFunctionType.Exp)

            for rb in range(N_RB):
                k, half = rb // 2, rb % 2      # batch-local, 0=stoch 1=det
                stoch = (half == 0)
                lsl = slice(half * 128, (half + 1) * 128)

                s0c = st.tile([128, N_CHUNK], f32, tag="s0c")
                s1c = st.tile([128, N_CHUNK], f32, tag="s1c")
                m8c = st.tile([128, N_CHUNK, 8], f32, tag="m8c")
                i8c = st.tile([128, N_CHUNK, 8], u32, tag="i8c")

                r_cks = []
                for ch in range(N_CHUNK):
                    csl = slice(ch * CHUNK, (ch + 1) * CHUNK)
                    a_ps = ps.tile([128, CHUNK], f32)
                    # lhsT-stationary ordering: LDWEIGHTS shared across wj/j4
                    groups = [(0, z0_sb, (w0_sb,)),
                              (1, z0_sb, (w0_sb,))]
                    n_sub = CHUNK // 512
                    nmm_slice = 2 + (2 if with_bias else 0)
                    cnt = [0] * n_sub
                    for (kk, zz, wws) in groups:
                        for ww in wws:
                            for j4 in range(n_sub):
                                nsl = slice(ch * CHUNK + j4 * 512,
                                            ch * CHUNK + (j4 + 1) * 512)
                                psl = slice(j4 * 512, (j4 + 1) * 512)
                                nc.tensor.matmul(
                                    a_ps[:, psl],
                                    zz[:, kk, k, lsl],
                                    ww[:, kk, nsl],
                                    start=(cnt[j4] == 0),
                                    stop=(cnt[j4] == nmm_slice - 1))
                                cnt[j4] += 1
                    if with_bias:
                        for pbs in (pb0_sb, pb1_sb):
                            for j4 in range(n_sub):
                                nsl = slice(ch * CHUNK + j4 * 512,
                                            ch * CHUNK + (j4 + 1) * 512)
                                psl = slice(j4 * 512, (j4 + 1) * 512)
                                nc.tensor.matmul(
                                    a_ps[:, psl], ones_sb, pbs[:, nsl],
                                    start=False,
                                    stop=(cnt[j4] == nmm_slice - 1))
                                cnt[j4] += 1
                    # value row chunk first: a (det rows scan PSUM
                    # directly) or a+gumbel (stoch; accumulate-DMA gets a
                    # full chunk of slack before the deferred scans read it)
                    if stoch:
                        r_ck = ck.tile([128, CHUNK], f32, tag="r_ck")
                        nc.scalar.activation(r_ck, a_ps,
                                             mybir.ActivationFunctionType.Copy)
                        nc.gpsimd.dma_start(out=r_ck, in_=gp[k, :, csl],
                                            accum_op=mybir.AluOpType.add)
                    else:
                        r_ck = a_ps
                    # exp(a - mhat) with running S0 accumulator
                    e_ck = ck.tile([128, CHUNK], bf16, tag="e_ck")
                    nc.scalar.activation(e_ck, a_ps,
                                         mybir.ActivationFunctionType.Exp,
                                         bias=ebias[:, 0:1], scale=1.0,
                                         accum_out=s0c[:, ch:ch + 1])
                    # S1 accumulator: sum a * exp(a - mhat)
                    scr = ck.tile([128, CHUNK], bf16, tag="scr")
                    nc.vector.scalar_tensor_tensor(
                        out=scr, in0=a_ps, scalar=0.0, in1=e_ck,
                        op0=mybir.AluOpType.add, op1=mybir.AluOpType.mult,
                        accum_out=s1c[:, ch:ch + 1])
                    r_cks.append(r_ck)
                    # deferred scans: stoch rows by 2 chunks (gumbel-DMA
                    # slack; SBUF tiles so PSUM isn't held), det rows by 1
                    defer = 2 if stoch else 1
                    if ch >= defer:
                        pc = ch - defer
                        nc.vector.max(m8c[:, pc], r_cks[pc])
                        nc.vector.max_index(i8c[:, pc], m8c[:, pc], r_cks[pc])

                for pc in range(N_CHUNK - defer, N_CHUNK):
                    nc.vector.max(m8c[:, pc], r_cks[pc])
                    nc.vector.max_index(i8c[:, pc], m8c[:, pc], r_cks[pc])

                nc.sync.dma_start(out=idx_out[rb], in_=i8c)
                s0r = sm.tile([128, 1], f32, tag="s0r")
                nc.vector.reduce_sum(s0r, s0c, axis=mybir.AxisListType.X)
                nc.sync.dma_start(out=st_out[rb, 0], in_=s0r)
                s1r = sm.tile([128, 1], f32, tag="s1r")
                nc.vector.reduce_sum(s1r, s1c, axis=mybir.AxisListType.X)
                nc.sync.dma_start(out=st_out[rb, 1], in_=s1r)
    nc.compile()
    return nc


_NC_CACHE = {}


def kernel(z, proj_w, proj_b, embed_w, gumbel, mask_noise, _trace=False):
    z = np.asarray(z, dtype=np.float32)
    proj_w = np.asarray(proj_w, dtype=np.float32)
    proj_b = np.asarray(proj_b, dtype=np.float32)
    embed_w = np.asarray(embed_w, dtype=np.float32)
    gumbel = np.asarray(gumbel, dtype=np.float32)
    mask_noise = np.asarray(mask_noise, dtype=np.float32)

    # --- host prep -------------------------------------------------------
    # mask, replicating jnp.argsort (stable) semantics exactly
    ids_shuffle = np.argsort(mask_noise, axis=1, kind="stable")
    ids_restore = np.argsort(ids_shuffle, axis=1, kind="stable")
    base = (np.arange(L) >= LEN_KEEP)
    mask = np.take_along_axis(np.broadcast_to(base, (B, L)), ids_restore,
                              axis=1)                      # (B, L) bool
    # per-batch position permutation: masked first, unmasked second
    perm = np.empty((B, L), dtype=np.int64)
    for b in range(B):
        m = mask[b]
        perm[b] = np.concatenate([np.where(m)[0], np.where(~m)[0]])

    zf = z.reshape(B, NH, L)
    zp = np.take_along_axis(zf, perm[:, None, :], axis=2)  # (B, NH, L)
    z0 = zp.astype(_bf16)
    def _zlay(a):  # (BPC', NH, L) -> (128, 2, BPC', L)
        return np.ascontiguousarray(
            a.reshape(a.shape[0], 2, 128, L).transpose(2, 1, 0, 3))

    w0 = proj_w.astype(_bf16)
    def _wlay(w):  # (NE, NH) -> (128, 2, NE)
        return np.ascontiguousarray(w.T.reshape(2, 128, NE).transpose(1, 0, 2))
    w0t = _wlay(w0)

    gf = gumbel.reshape(B, NE, L)
    gperm = np.empty((B, 128, NE), dtype=np.float32)
    for b in range(B):
        gperm[b] = gf[b][:, perm[b][:LEN_KEEP]].T          # (128, NE)

    with_bias = bool(np.any(proj_b != 0.0))
    pb0 = proj_b.astype(_bf16)[None, :]
    pb1 = (proj_b - pb0[0].astype(np.float32)).astype(_bf16)[None, :]

    # exp bias: safe upper bound on |logits| (Cauchy-Schwarz), rounded for
    # program-cache stability
    zn = np.sqrt((zf.astype(np.float64) ** 2).sum(axis=1)).max()
    wn = np.sqrt((proj_w.astype(np.float64) ** 2).sum(axis=1)).max()
    mhat = float(np.ceil(zn * wn + np.abs(proj_b).max() + 1.0))

    key = (mhat, with_bias)
    if key not in _NC_CACHE:
        _NC_CACHE[key] = _build(mhat, with_bias)
    nc = _NC_CACHE[key]

    in_maps = []
    for c in range(N_CORES):
        bs = slice(c * BPC, (c + 1) * BPC)
        m = dict(z0=_zlay(z0[bs]), w0t=w0t, gp=gperm[bs])
        if with_bias:
            m["pb0"] = pb0
            m["pb1"] = pb1
        in_maps.append(m)

    res = run_bass_kernel_spmd(nc, in_maps, list(range(N_CORES)),
                               trace=_trace)

    # --- host assembly ---------------------------------------------------
    ind = np.empty((B, L), dtype=np.int32)
    s0_all = np.empty((B, L), dtype=np.float64)
    s1_all = np.empty((B, L), dtype=np.float64)
    for c in range(N_CORES):
        r = res.results[c]
        for k in range(BPC):
            b = c * BPC + k
            # exact rescore of the stage-1 candidates (<=64 per position)
            chunk_base = (np.arange(N_CHUNK, dtype=np.int64) * CHUNK)[None, :, None]
            parts = []
            for li, rbl in ((0, 2 * k), (1, 2 * k + 1)):
                cand = (r["idx_out"][rbl].astype(np.int64)
                        + chunk_base).reshape(128, N_CHUNK * 8)
                wc = proj_w[cand]                    # (128, 64, NH) f32
                zc = zp[b, :, li * 128:(li + 1) * 128]       # (NH, 128)
                vals = np.einsum('pkc,cp->pk', wc.astype(np.float64),
                                 zc.astype(np.float64))
                vals += proj_b.astype(np.float64)[cand]
                if li == 0:                          # stoch: add gumbel
                    lpos = perm[b][:LEN_KEEP]
                    vals += gf[b][cand, lpos[:, None]].astype(np.float64)
                best = vals.argmax(axis=1)
                parts.append(cand[np.arange(128), best])
            ind[b, perm[b]] = np.concatenate(parts).astype(np.int32)
            s0p = np.concatenate([r["st_out"][2 * k, 0],
                                  r["st_out"][2 * k + 1, 0]])
            s1p = np.concatenate([r["st_out"][2 * k, 1],
                                  r["st_out"][2 * k + 1, 1]])
            s0_all[b, perm[b]] = s0p
            s1_all[b, perm[b]] = s1p

    kl = s1_all / s0_all - (mhat + np.log(s0_all)) + np.log(NE)
    prior_loss = np.float32(KLW * kl.mean())

    z_q = embed_w[ind]                                     # (B, L, ED)
    z_q = np.ascontiguousarray(z_q.transpose(0, 2, 1)).reshape(B, ED, HWD, HWD)

    ind = ind.reshape(B, HWD, HWD)
    if _trace:
        return (z_q, prior_loss, ind), res
    return z_q, prior_loss, ind
